# revision 1
# baseline (speedup 1.0000x reference)
"""Trainium2 Bass kernel for nn_ButterflyFilter.

The reference applies, per length-512 row (flattened b*c*angles):
  zero-pad to 1024 -> 10-stage butterfly "FFT" (stage order decreasing)
  -> elementwise filter (bit-reversed order) -> 10-stage butterfly
  "IFFT" (stage order increasing) -> real part of first 512 entries.

Every step is linear in x, so the whole chain is one complex 1024x1024
operator A determined by (twiddle_fft, twiddle_ifft, fourier_filter_br).
Since x is real with support on [:512] and only Re(y)[:512] is kept, the
effective map is the real 512x512 matrix W = Re(A)[:512, :512]:

    proj_row = W @ x_row

x in HBM is (b, c, s, a) — for fixed (b, c) the tile is (s, a), i.e. rows
(angles) are already laid out column-major, exactly the moving-operand
layout the TensorEngine wants. So the device work is 16 independent
512x512x512 matmuls out_bc = W @ x_bc, data-parallel 2 per core across
8 cores. The small parameter-folding (building W from the twiddles) runs
on host in float64; the 32 MiB of row data never touches the host math.
"""

import os
import sys
import types
from contextlib import ExitStack

import numpy as np

import concourse.bass as bass
import concourse.mybir as mybir
from concourse.bass_utils import run_bass_kernel_spmd


def _ensure_axon_hooks():
    # concourse.bass_utils imports antenv.axon_hooks on the trace path; some
    # images lack that module. Provide a no-op holder so a BASS_TRACE env set
    # by the caller can't crash the run.
    try:
        import antenv.axon_hooks  # noqa: F401
    except Exception:
        m = types.ModuleType("antenv.axon_hooks")
        m._h = None
        m.set_axon_ntff_profile_hook = lambda h: setattr(m, "_h", h)
        m.get_axon_ntff_profile_hook = lambda: m._h
        sys.modules["antenv.axon_hooks"] = m


_ensure_axon_hooks()

N_CORES = 8
S = 512          # input/output row length
NF = 1024        # padded length
P = 128          # SBUF partitions
BC_PER_CORE = 2  # 16 (b,c) tiles / 8 cores

# Exposed for the test harness: exec time of the last device run (ns), if
# profiling was enabled via BUTTERFLY_TRACE=1.
last_exec_time_ns = None
last_results = None


def _butterfly_np(tw, x, increasing):
    # Mirrors the reference butterfly exactly, in numpy (any dtype).
    B, n = x.shape
    m = tw.shape[0]
    order = range(m) if increasing else range(m - 1, -1, -1)
    for idx in order:
        s = 1 << idx
        t = tw[idx].reshape(n // (2 * s), s, 2, 2)
        xr = x.reshape(B, n // (2 * s), 2, s)
        x = np.einsum('gjik,bgkj->bgij', t, xr).reshape(B, n)
    return x


def _compose_wt(twiddle_fft, twiddle_ifft, fourier_filter_br):
    """Fold twiddles+filter into the lhsT operand Wt[i_in, o_out] (512x512 f32)."""
    tw_fft = np.asarray(twiddle_fft, dtype=np.float64)
    tw_ifft = np.asarray(twiddle_ifft, dtype=np.float64)
    filt = np.asarray(fourier_filter_br, dtype=np.float64)
    tf = tw_fft[0, ..., 0] + 1j * tw_fft[0, ..., 1]
    ti = tw_ifft[0, ..., 0] + 1j * tw_ifft[0, ..., 1]
    X = np.eye(NF, dtype=np.complex128)      # row j = e_j
    X = _butterfly_np(tf, X, increasing=False)
    X = X * filt[None, :]
    X = _butterfly_np(ti, X, increasing=True)
    # X = chain(I) = A^T, so X[i, o] = A[o, i]; W[o, i] = Re(A[o, i]).
    # lhsT for out = lhsT.T @ rhs must be Wt[i, o] = W[o, i] = Re(X[i, o]).
    return np.ascontiguousarray(np.real(X[:S, :S]).astype(np.float32))


def _mm_dtype():
    return (
        mybir.dt.float32r
        if os.environ.get("BUTTERFLY_MM_DTYPE", "fp32r") == "fp32r"
        else mybir.dt.float32
    )


def _build_nc():
    # Raw Bass (no TileContext): this walrus encodes at most ONE semaphore
    # wait per instruction, which Tile's scheduler and epilogue drain cannot
    # guarantee. With manual engine programs every wait is its own wait_ge.
    #
    # Layout (per core):
    #   wx[k] (128, 1024) = [W_k | x0_k]: contraction chunk k of the operator
    #   fused with bc-tile-0's chunk, one 512 KiB DMA piece each, so compute
    #   starts on the first piece. x1[k] (128, 512) are bc-tile-1's chunks.
    #   out_bc[o*128+p, a] accumulates in one PSUM bank per (bc, o) group,
    #   is copied to SBUF by DVE, and stored as 256 KiB contiguous chunks.
    mmdt = _mm_dtype()
    kc = S // P  # 4 contraction chunks
    oc = S // P  # 4 output-row chunks
    f32 = mybir.dt.float32
    # PE warm-up matmuls (HAM un-throttle) during the input DMA wait. Each
    # fp32 matmul emits 2 HW passes at ~640 ns cold, so 3 calls ~= 3.8 us of
    # dense PE busy — enough to trip HAM's ~3.4 us SHORT window right as the
    # first input piece lands (measured: 2 calls leave the real stream cold).
    n_warm = 3

    nc = bass.Bass()
    wx = nc.declare_dram_parameter("wx", [kc, P, 2 * S], mmdt, isOutput=False)
    x1d = nc.declare_dram_parameter("x1", [kc, P, S], mmdt, isOutput=False)
    out = nc.declare_dram_parameter("out", [BC_PER_CORE, S, S], f32, isOutput=True)

    with ExitStack() as ctx:
        wx_sb = [
            ctx.enter_context(nc.sbuf_tensor(f"wx_sb{k}", [P, 2 * S], mmdt))
            for k in range(kc)
        ]
        x1_sb = ctx.enter_context(nc.sbuf_tensor("x1_sb", [P, 4 * S], mmdt))
        warm_sb = ctx.enter_context(nc.sbuf_tensor("warm_sb", [P, 3 * P + 32], f32))
        o_sb = [
            ctx.enter_context(nc.sbuf_tensor(f"o_sb{j}", [P, 4 * S], f32))
            for j in range(2)
        ]
        accs = [
            ctx.enter_context(nc.psum_tensor(f"acc{g}", [P, S], f32))
            for g in range(BC_PER_CORE * oc)
        ]
        s_wx = [ctx.enter_context(nc.semaphore(f"s_wx{k}")) for k in range(kc)]
        s_x1 = [ctx.enter_context(nc.semaphore(f"s_x1{k}")) for k in range(kc)]
        s_warm = ctx.enter_context(nc.semaphore("s_warm"))
        s_pe = ctx.enter_context(nc.semaphore("s_pe"))
        s_dve = ctx.enter_context(nc.semaphore("s_dve"))
        s_cpa = ctx.enter_context(nc.semaphore("s_cpa"))
        s_out = ctx.enter_context(nc.semaphore("s_out"))
        block = ctx.enter_context(nc.Block())

        @block.sync
        def _(sync):
            # Input pieces, issue order = consumption order. 512 KiB each for
            # wx (W chunk fused with bc0 x chunk), 256 KiB each for x1.
            for k in range(kc):
                sync.dma_start(wx_sb[k][:], wx[k]).then_inc(s_wx[k], 16)
            for k in range(kc):
                sync.dma_start(x1_sb[:, bass.ts(k, S)], x1d[k]).then_inc(s_x1[k], 16)
            sync.wait_ge(s_out, BC_PER_CORE * oc * 16)

        @block.tensor
        def _(tensor):
            # Warm-up matmuls on a zeroed scratch tile: keeps the PE busy
            # while inputs stream in so HAM un-throttles (1.2 -> 2.4 GHz)
            # before the real matmuls. Results land in acc 7 which is cleared
            # by its real accumulation group's start=True much later.
            tensor.wait_ge(s_warm, 1)
            for _ in range(n_warm):
                nc.tensor.matmul(
                    accs[-1][:, : 2 * P], warm_sb[:, :P], warm_sb[:, P : 3 * P],
                    start=True, stop=True,
                )
            # bc0: k-outer so compute starts on the first 512 KiB piece.
            for k in range(kc):
                tensor.wait_ge(s_wx[k], 16)
                for o in range(oc):
                    mm = nc.tensor.matmul(
                        accs[o][:],
                        wx_sb[k][:, bass.ts(o, P)],
                        wx_sb[k][:, S : 2 * S],
                        start=(k == 0),
                        stop=(k == kc - 1),
                    )
                    if k == kc - 1:
                        mm.then_inc(s_pe, 1)
            # bc1
            for k in range(kc):
                tensor.wait_ge(s_x1[k], 16)
                for o in range(oc):
                    mm = nc.tensor.matmul(
                        accs[oc + o][:],
                        wx_sb[k][:, bass.ts(o, P)],
                        x1_sb[:, bass.ts(k, S)],
                        start=(k == 0),
                        stop=(k == kc - 1),
                    )
                    if k == kc - 1:
                        mm.then_inc(s_pe, 1)

        @block.vector
        def _(vector):
            nc.vector.memset(warm_sb[:], 0.0).then_inc(s_warm, 1)
            for g in range(BC_PER_CORE * oc):
                bc, o = divmod(g, oc)
                vector.wait_ge(s_pe, g + 1)
                nc.vector.tensor_copy(
                    o_sb[bc][:, bass.ts(o, S)], accs[g][:]
                ).then_inc(s_dve, 1)

        @block.scalar
        def _(scalar):
            # Per-group 256 KiB stores from the otherwise-idle ACT engine so
            # output drains as soon as each o-chunk is copied out of PSUM.
            for g in range(BC_PER_CORE * oc):
                bc, o = divmod(g, oc)
                scalar.wait_ge(s_dve, g + 1)
                scalar.dma_start(
                    out[bc, bass.ts(o, P), :], o_sb[bc][:, bass.ts(o, S)]
                ).then_inc(s_out, 16)

    return nc


def kernel(x, twiddle_fft, twiddle_ifft, fourier_filter_br):
    global last_exec_time_ns, last_results
    x = np.asarray(x, dtype=np.float32)
    b, c, s_len, a = x.shape
    assert (b, c, s_len, a) == (8, 2, S, S)

    wt = _compose_wt(twiddle_fft, twiddle_ifft, fourier_filter_br)
    x16 = x.reshape(b * c, S // P, P, S)  # [bc, k, p, m]
    wt4 = wt.reshape(S // P, P, S)

    in_maps = []
    for core in range(N_CORES):
        x0 = x16[BC_PER_CORE * core]
        x1 = x16[BC_PER_CORE * core + 1]
        # wx[k] = [w_k | x0_k] along the free dim, one 512 KiB DMA piece each
        wx = np.concatenate([wt4, x0], axis=2)  # (4, 128, 1024)
        in_maps.append(
            {
                "wx": np.ascontiguousarray(wx),
                "x1": np.ascontiguousarray(x1),
            }
        )
    nc = _build_nc()
    trace = os.environ.get("BUTTERFLY_TRACE") == "1"
    res = run_bass_kernel_spmd(nc, in_maps, core_ids=list(range(N_CORES)), trace=trace)
    last_exec_time_ns = res.exec_time_ns
    last_results = res

    q = np.concatenate([res.results[k]["out"] for k in range(N_CORES)], axis=0)
    # q[bc, o, a] = proj.T[o, bc*512 + a]; reference output is
    # proj.T.reshape(b, c, s, a) — a pure reinterpret of the (512, 8192) buffer.
    out = q.transpose(1, 0, 2).reshape(S, b * c * a).reshape(b, c, s_len, a)
    return np.ascontiguousarray(out).astype(np.float32)



# revision 5
# speedup vs baseline: 1.3180x; 1.3180x over previous
"""Trainium2 Bass kernel for nn_ButterflyFilter.

The reference applies, per length-512 row (flattened b*c*angles):
  zero-pad to 1024 -> 10-stage butterfly "FFT" (stage order decreasing)
  -> elementwise filter (bit-reversed order) -> 10-stage butterfly
  "IFFT" (stage order increasing) -> real part of first 512 entries.

Every step is linear in x, so the whole chain is one complex 1024x1024
operator A determined by (twiddle_fft, twiddle_ifft, fourier_filter_br).
Since x is real with support on [:512] and only Re(y)[:512] is kept, the
effective map is the real 512x512 matrix W = Re(A)[:512, :512]:

    proj_row = W @ x_row

For the actual FBP ramp-filter parameters, W is a circular-convolution
(Toeplitz) matrix whose kernel decays as 1/k^2: truncating it to a band
of half-width 64 costs ~1.7e-4 relative error (gate is 2e-2). Each
128-row output chunk b then only needs contraction rows
[128b-64, 128b+192), i.e. TWO 128-row windows from the offset-64
partitioning of the input rows -- 2 matmuls per chunk instead of 4, in
bf16 (~2.6e-3 total err). The device work per core is 2 (b,c) tiles x
4 output chunks x 2 matmuls. If the runtime-composed W turns out not to
be banded (e.g. random twiddles), we fall back to the exact dense fp32r
path automatically.
"""

import os
import sys
import types
from contextlib import ExitStack

import numpy as np

import concourse.bass as bass
import concourse.mybir as mybir
from concourse.bass_utils import run_bass_kernel_spmd


def _ensure_axon_hooks():
    # concourse.bass_utils imports antenv.axon_hooks on the trace path; some
    # images lack that module. Provide a no-op holder so a BASS_TRACE env set
    # by the caller can't crash the run.
    try:
        import antenv.axon_hooks  # noqa: F401
    except Exception:
        m = types.ModuleType("antenv.axon_hooks")
        m._h = None
        m.set_axon_ntff_profile_hook = lambda h: setattr(m, "_h", h)
        m.get_axon_ntff_profile_hook = lambda: m._h
        sys.modules["antenv.axon_hooks"] = m


_ensure_axon_hooks()

N_CORES = 8
S = 512          # input/output row length
NF = 1024        # padded length
P = 128          # SBUF partitions
BC_PER_CORE = 2  # 16 (b,c) tiles / 8 cores

# Band scheme geometry: input rows are partitioned at the offset-64
# boundaries; chunk c0=[0,64) and c4=[448,512) share one 128-partition
# SBUF block (c0 in partitions 0:64, c4 in 64:128).
CHUNK_BOUNDS = [0, 64, 192, 320, 448, 512]
# Per output chunk b: two matmuls (wcol, partition range, x block).
BAND_PLAN = {
    0: [(0, (0, 64), 0), (1, (0, 128), 1)],
    1: [(2, (0, 128), 1), (3, (0, 128), 2)],
    2: [(4, (0, 128), 2), (5, (0, 128), 3)],
    3: [(6, (0, 128), 3), (0, (64, 128), 0)],
}
BAND_ERR_MAX = 5e-3  # operator-level truncation error gate for band path

# slabA column layout (bf16): W column blocks + bc0's x blocks, ordered so
# chunk b0's operands arrive in the first DMA piece.
#   [wcol0 | wcol1 | xblk0 | xblk1 | wcol2..wcol6 | xblk2 | xblk3]
A_COLS = 2944
B_COLS = 2048
# DMA piece boundaries (cols) and the piece count each output chunk needs.
A_PIECES = [(0, 1280), (1280, 2432), (2432, 2944)]
A_THRESH = [1, 2, 3, 3]  # chunk b of bc0 waits for this many A pieces
B_PIECES = [(0, 1024), (1024, 1536), (1536, 2048)]
B_THRESH = [1, 2, 3, 3]

WCOL_OFF = {0: 0, 1: 128, 2: 1280, 3: 1408, 4: 1536, 5: 1664, 6: 1792}
XBLK_OFF_A = {0: 256, 1: 768, 2: 1920, 3: 2432}
XBLK_OFF_B = {0: 0, 1: 512, 2: 1024, 3: 1536}

# Exposed for the test harness: exec time of the last device run (ns), if
# profiling was enabled via BUTTERFLY_TRACE=1.
last_exec_time_ns = None
last_results = None
last_path = None  # "band" or "dense", for the harness/debugging


def _bf16():
    return mybir.dt.np(mybir.dt.bfloat16)


def _butterfly_np(tw, x, increasing):
    # Mirrors the reference butterfly exactly, in numpy (any dtype).
    B, n = x.shape
    m = tw.shape[0]
    order = range(m) if increasing else range(m - 1, -1, -1)
    for idx in order:
        s = 1 << idx
        t = tw[idx].reshape(n // (2 * s), s, 2, 2)
        xr = x.reshape(B, n // (2 * s), 2, s)
        x = np.einsum('gjik,bgkj->bgij', t, xr).reshape(B, n)
    return x


def _compose_wt(twiddle_fft, twiddle_ifft, fourier_filter_br):
    """Fold twiddles+filter into the lhsT operand Wt[i_in, o_out] (512x512 f32)."""
    tw_fft = np.asarray(twiddle_fft, dtype=np.float64)
    tw_ifft = np.asarray(twiddle_ifft, dtype=np.float64)
    filt = np.asarray(fourier_filter_br, dtype=np.float64)
    tf = tw_fft[0, ..., 0] + 1j * tw_fft[0, ..., 1]
    ti = tw_ifft[0, ..., 0] + 1j * tw_ifft[0, ..., 1]
    X = np.eye(NF, dtype=np.complex128)      # row j = e_j
    X = _butterfly_np(tf, X, increasing=False)
    X = X * filt[None, :]
    X = _butterfly_np(ti, X, increasing=True)
    # X = chain(I) = A^T, so X[i, o] = A[o, i]; W[o, i] = Re(A[o, i]).
    # lhsT for out = lhsT.T @ rhs must be Wt[i, o] = W[o, i] = Re(X[i, o]).
    return np.ascontiguousarray(np.real(X[:S, :S]).astype(np.float32))


def _band_error(wt):
    """Relative Frobenius mass of W outside the 2-window band cover."""
    cov = np.zeros((S, S), bool)  # indexed [i, o] like wt
    for b in range(4):
        lo, hi = max(0, 128 * b - 64), min(S, 128 * b + 192)
        cov[lo:hi, 128 * b:128 * b + 128] = True
    tot = float(np.square(wt).sum())
    off = float(np.square(wt[~cov]).sum())
    return (off / tot) ** 0.5 if tot > 0 else 0.0


def _band_wcols(wt):
    wc = np.zeros((7, P, P), np.float32)
    wc[0][0:64] = wt[0:64, 0:128]          # b0 j0 (chunk c0)
    wc[0][64:128] = wt[448:512, 384:512]   # b3 j1 (chunk c4)
    wc[1] = wt[64:192, 0:128]    # b0 j1
    wc[2] = wt[64:192, 128:256]  # b1 j0
    wc[3] = wt[192:320, 128:256]  # b1 j1
    wc[4] = wt[192:320, 256:384]  # b2 j0
    wc[5] = wt[320:448, 256:384]  # b2 j1
    wc[6] = wt[320:448, 384:512]  # b3 j0
    return wc


def _band_xblocks(xbc):
    bl = np.zeros((4, P, S), np.float32)
    bl[0][0:64] = xbc[0:64]
    bl[0][64:128] = xbc[448:512]
    bl[1] = xbc[64:192]
    bl[2] = xbc[192:320]
    bl[3] = xbc[320:448]
    return bl


def _build_nc_band():
    # Raw Bass (no TileContext); one semaphore wait per instruction.
    #
    # Per core: slabA (128, 2944) bf16 = [W tiles | bc0 x blocks] streamed in
    # 3 pieces on the sync queue; slabB (128, 2048) bf16 = bc1 x blocks in 3
    # pieces on the scalar queue (the two HWDGE queues stream in parallel).
    # 16 bf16 matmuls (2 per (bc, output-chunk) group) accumulate into 8 PSUM
    # banks; PSUM->SBUF bf16 downcast copies alternate between DVE (even
    # groups) and ACT (odd groups; GpSimd has no PSUM port) so the drain
    # keeps pace with the PE; paired 256 KiB stores go back out on the sync
    # queue, which is idle once the input pieces are issued.
    bf = mybir.dt.bfloat16
    f32 = mybir.dt.float32
    n_warm = 4  # PE warm-up matmuls (HAM un-throttle) during the input wait

    nc = bass.Bass()
    slab_a = nc.declare_dram_parameter("slab_a", [P, A_COLS], bf, isOutput=False)
    slab_b = nc.declare_dram_parameter("slab_b", [P, B_COLS], bf, isOutput=False)
    out = nc.declare_dram_parameter("out", [BC_PER_CORE, P, 4 * S], bf, isOutput=True)

    with ExitStack() as ctx:
        a_sb = ctx.enter_context(nc.sbuf_tensor("a_sb", [P, A_COLS], bf))
        b_sb = ctx.enter_context(nc.sbuf_tensor("b_sb", [P, B_COLS], bf))
        warm_sb = ctx.enter_context(nc.sbuf_tensor("warm_sb", [P, 3 * P + 32], f32))
        o_sb = [
            ctx.enter_context(nc.sbuf_tensor(f"o_sb{j}", [P, 4 * S], bf))
            for j in range(BC_PER_CORE)
        ]
        accs = [
            ctx.enter_context(nc.psum_tensor(f"acc{g}", [P, S], f32))
            for g in range(BC_PER_CORE * 4)
        ]
        s_a = ctx.enter_context(nc.semaphore("s_a"))
        s_b = ctx.enter_context(nc.semaphore("s_b"))
        s_warm = ctx.enter_context(nc.semaphore("s_warm"))
        s_pe = ctx.enter_context(nc.semaphore("s_pe"))
        s_cv = ctx.enter_context(nc.semaphore("s_cv"))
        s_cp = ctx.enter_context(nc.semaphore("s_cp"))
        s_out = ctx.enter_context(nc.semaphore("s_out"))
        block = ctx.enter_context(nc.Block())

        def xsb(bc, blk):
            base = XBLK_OFF_A[blk] if bc == 0 else XBLK_OFF_B[blk]
            sb = a_sb if bc == 0 else b_sb
            return sb, base

        @block.sync
        def _(sync):
            for lo, hi in A_PIECES:
                sync.dma_start(a_sb[:, lo:hi], slab_a[:, lo:hi]).then_inc(s_a, 16)
            # Paired stores: pair k covers groups {2k, 2k+1} = the (k+1)-th
            # copy on each of DVE (even) and ACT (odd).
            for k in range(4):
                bc, pair = divmod(k, 2)
                sync.wait_ge(s_cv, k + 1)
                sync.wait_ge(s_cp, k + 1)
                sync.dma_start(
                    out[bc, :, 1024 * pair: 1024 * pair + 1024],
                    o_sb[bc][:, 1024 * pair: 1024 * pair + 1024],
                ).then_inc(s_out, 16)
            sync.wait_ge(s_out, 4 * 16)

        @block.tensor
        def _(tensor):
            # Warm-up matmuls on a zeroed scratch tile keep the PE busy while
            # inputs stream in, so HAM grants full clock as real work starts.
            tensor.wait_ge(s_warm, 1)
            for _ in range(n_warm):
                nc.tensor.matmul(
                    accs[-1][:, : 2 * P], warm_sb[:, :P], warm_sb[:, P: 3 * P],
                    start=True, stop=True,
                )
            for bc in range(BC_PER_CORE):
                sem, thresh = (s_a, A_THRESH) if bc == 0 else (s_b, B_THRESH)
                for b in range(4):
                    tensor.wait_ge(sem, thresh[b] * 16)
                    for j, (w, (p0, p1), blk) in enumerate(BAND_PLAN[b]):
                        sb, base = xsb(bc, blk)
                        mm = nc.tensor.matmul(
                            accs[4 * bc + b][:],
                            a_sb[p0:p1, WCOL_OFF[w]: WCOL_OFF[w] + P],
                            sb[p0:p1, base: base + S],
                            start=(j == 0),
                            stop=(j == 1),
                        )
                        if j == 1:
                            mm.then_inc(s_pe, 1)

        @block.vector
        def _(vector):
            nc.vector.memset(warm_sb[:], 0.0).then_inc(s_warm, 1)
            for g in range(0, BC_PER_CORE * 4, 2):
                bc, b = divmod(g, 4)
                vector.wait_ge(s_pe, g + 1)
                nc.vector.tensor_copy(
                    o_sb[bc][:, bass.ts(b, S)], accs[g][:]
                ).then_inc(s_cv, 1)

        @block.scalar
        def _(scalar):
            for lo, hi in B_PIECES:
                scalar.dma_start(b_sb[:, lo:hi], slab_b[:, lo:hi]).then_inc(s_b, 16)
            for g in range(1, BC_PER_CORE * 4, 2):
                bc, b = divmod(g, 4)
                scalar.wait_ge(s_pe, g + 1)
                nc.scalar.copy(
                    o_sb[bc][:, bass.ts(b, S)], accs[g][:]
                ).then_inc(s_cp, 1)

    return nc


def _run_band(x16, wt, trace):
    bf16 = _bf16()
    wc = _band_wcols(wt).astype(bf16)
    in_maps = []
    for core in range(N_CORES):
        bl0 = _band_xblocks(x16[BC_PER_CORE * core]).astype(bf16)
        bl1 = _band_xblocks(x16[BC_PER_CORE * core + 1]).astype(bf16)
        slab_a = np.concatenate(
            [wc[0], wc[1], bl0[0], bl0[1], wc[2], wc[3], wc[4], wc[5], wc[6],
             bl0[2], bl0[3]],
            axis=1,
        )
        slab_b = np.concatenate([bl1[0], bl1[1], bl1[2], bl1[3]], axis=1)
        in_maps.append(
            {
                "slab_a": np.ascontiguousarray(slab_a),
                "slab_b": np.ascontiguousarray(slab_b),
            }
        )
    nc = _build_nc_band()
    res = run_bass_kernel_spmd(nc, in_maps, core_ids=list(range(N_CORES)), trace=trace)
    # out[bc, p, 512*b + a] -> q[bc_global, 128*b + p, a]
    q = np.concatenate(
        [
            np.asarray(res.results[k]["out"], dtype=np.float32)
            .reshape(BC_PER_CORE, P, 4, S)
            .transpose(0, 2, 1, 3)
            .reshape(BC_PER_CORE, S, S)
            for k in range(N_CORES)
        ],
        axis=0,
    )
    return q, res


# ---------------------------------------------------------------------------
# Dense fallback (exact, fp32r) -- the original data layout: 16 matmuls/core.
# ---------------------------------------------------------------------------

def _build_nc_dense():
    mmdt = mybir.dt.float32r
    kc = S // P  # 4 contraction chunks
    oc = S // P  # 4 output-row chunks
    f32 = mybir.dt.float32
    n_warm = 3

    nc = bass.Bass()
    wx = nc.declare_dram_parameter("wx", [kc, P, 2 * S], mmdt, isOutput=False)
    x1d = nc.declare_dram_parameter("x1", [kc, P, S], mmdt, isOutput=False)
    out = nc.declare_dram_parameter("out", [BC_PER_CORE, S, S], f32, isOutput=True)

    with ExitStack() as ctx:
        wx_sb = [
            ctx.enter_context(nc.sbuf_tensor(f"wx_sb{k}", [P, 2 * S], mmdt))
            for k in range(kc)
        ]
        x1_sb = ctx.enter_context(nc.sbuf_tensor("x1_sb", [P, 4 * S], mmdt))
        warm_sb = ctx.enter_context(nc.sbuf_tensor("warm_sb", [P, 3 * P + 32], f32))
        o_sb = [
            ctx.enter_context(nc.sbuf_tensor(f"o_sb{j}", [P, 4 * S], f32))
            for j in range(2)
        ]
        accs = [
            ctx.enter_context(nc.psum_tensor(f"acc{g}", [P, S], f32))
            for g in range(BC_PER_CORE * oc)
        ]
        s_wx = [ctx.enter_context(nc.semaphore(f"s_wx{k}")) for k in range(kc)]
        s_x1 = [ctx.enter_context(nc.semaphore(f"s_x1{k}")) for k in range(kc)]
        s_warm = ctx.enter_context(nc.semaphore("s_warm"))
        s_pe = ctx.enter_context(nc.semaphore("s_pe"))
        s_dve = ctx.enter_context(nc.semaphore("s_dve"))
        s_out = ctx.enter_context(nc.semaphore("s_out"))
        block = ctx.enter_context(nc.Block())

        @block.sync
        def _(sync):
            for k in range(kc):
                sync.dma_start(wx_sb[k][:], wx[k]).then_inc(s_wx[k], 16)
            for k in range(kc):
                sync.dma_start(x1_sb[:, bass.ts(k, S)], x1d[k]).then_inc(s_x1[k], 16)
            sync.wait_ge(s_out, BC_PER_CORE * oc * 16)

        @block.tensor
        def _(tensor):
            tensor.wait_ge(s_warm, 1)
            for _ in range(n_warm):
                nc.tensor.matmul(
                    accs[-1][:, : 2 * P], warm_sb[:, :P], warm_sb[:, P: 3 * P],
                    start=True, stop=True,
                )
            for k in range(kc):
                tensor.wait_ge(s_wx[k], 16)
                for o in range(oc):
                    mm = nc.tensor.matmul(
                        accs[o][:],
                        wx_sb[k][:, bass.ts(o, P)],
                        wx_sb[k][:, S: 2 * S],
                        start=(k == 0),
                        stop=(k == kc - 1),
                    )
                    if k == kc - 1:
                        mm.then_inc(s_pe, 1)
            for k in range(kc):
                tensor.wait_ge(s_x1[k], 16)
                for o in range(oc):
                    mm = nc.tensor.matmul(
                        accs[oc + o][:],
                        wx_sb[k][:, bass.ts(o, P)],
                        x1_sb[:, bass.ts(k, S)],
                        start=(k == 0),
                        stop=(k == kc - 1),
                    )
                    if k == kc - 1:
                        mm.then_inc(s_pe, 1)

        @block.vector
        def _(vector):
            nc.vector.memset(warm_sb[:], 0.0).then_inc(s_warm, 1)
            for g in range(BC_PER_CORE * oc):
                bc, o = divmod(g, oc)
                vector.wait_ge(s_pe, g + 1)
                nc.vector.tensor_copy(
                    o_sb[bc][:, bass.ts(o, S)], accs[g][:]
                ).then_inc(s_dve, 1)

        @block.scalar
        def _(scalar):
            for g in range(BC_PER_CORE * oc):
                bc, o = divmod(g, oc)
                scalar.wait_ge(s_dve, g + 1)
                scalar.dma_start(
                    out[bc, bass.ts(o, P), :], o_sb[bc][:, bass.ts(o, S)]
                ).then_inc(s_out, 16)

    return nc


def _run_dense(x16, wt, trace):
    x16k = x16.reshape(BC_PER_CORE * N_CORES, S // P, P, S)
    wt4 = wt.reshape(S // P, P, S)
    in_maps = []
    for core in range(N_CORES):
        x0 = x16k[BC_PER_CORE * core]
        x1 = x16k[BC_PER_CORE * core + 1]
        wx = np.concatenate([wt4, x0], axis=2)
        in_maps.append(
            {
                "wx": np.ascontiguousarray(wx),
                "x1": np.ascontiguousarray(x1),
            }
        )
    nc = _build_nc_dense()
    res = run_bass_kernel_spmd(nc, in_maps, core_ids=list(range(N_CORES)), trace=trace)
    q = np.concatenate(
        [np.asarray(res.results[k]["out"], dtype=np.float32) for k in range(N_CORES)],
        axis=0,
    )
    return q, res


def kernel(x, twiddle_fft, twiddle_ifft, fourier_filter_br):
    global last_exec_time_ns, last_results, last_path
    x = np.asarray(x, dtype=np.float32)
    b, c, s_len, a = x.shape
    assert (b, c, s_len, a) == (8, 2, S, S)

    wt = _compose_wt(twiddle_fft, twiddle_ifft, fourier_filter_br)
    x16 = x.reshape(b * c, S, S)  # [bc, row, angle]
    trace = os.environ.get("BUTTERFLY_TRACE") == "1"

    use_band = (
        os.environ.get("BUTTERFLY_FORCE_DENSE") != "1"
        and _band_error(wt) < BAND_ERR_MAX
    )
    if use_band:
        q, res = _run_band(x16, wt, trace)
        last_path = "band"
    else:
        q, res = _run_dense(x16, wt, trace)
        last_path = "dense"
    last_exec_time_ns = res.exec_time_ns
    last_results = res

    # q[bc, o, a] = proj.T[o, bc*512 + a]; reference output is
    # proj.T.reshape(b, c, s, a) — a pure reinterpret of the (512, 8192) buffer.
    out = q.transpose(1, 0, 2).reshape(S, b * c * a).reshape(b, c, s_len, a)
    return np.ascontiguousarray(out).astype(np.float32)
